# revision 1
# baseline (speedup 1.0000x reference)
"""CTRNN cell (RK4, 6 unfolds) as a Bass/Tile kernel on 8 Trainium2 cores.

Data-parallel: batch (32768) sharded 8 ways; weights replicated; no
cross-core communication. Per core: 4096 batch rows, 24 sequential
(4096x512)@(512x512) recurrent matmuls plus one (4096x256)@(256x512).

Layout: state kept transposed (units on partitions, batch on the free dim)
so h@R maps to lhsT=R-block (natural DRAM layout), rhs=state. Batch is
processed in chunks of 512 columns (one fp32 PSUM bank), in interleaved
groups of 4 so the tensor engine stays continuously busy (TRN2 p-state
ramp: matmuls run 2-3.7x slower unless PE stays hot) while other chunks'
element-wise tails run on DVE/ACT/Pool. Matmul order is
(unit-block, k-block, chunk) so each stationary weight serves the whole
group. xb = x@K + bias is injected into PSUM via an identity-weight
matmul, so tanh reads PSUM directly and no separate add pass exists.

Precision (state_dt="mixed", chosen from an error decomposition):
  - h accumulates in fp32 (bf16 h-storage alone costs 8e-3 relative
    error); h feeds no matmuls -- a bf16 shadow copy serves those and the
    element-wise ops (v-path quantization only costs ~7e-4).
  - everything else (weights, xb, tanh outputs u, RK4 increments d,
    stage inputs v) is bf16, keeping matmuls pure-bf16 at full PE rate
    and the hot DVE ops in their 2x/4x fast modes.
  - measured on hardware vs the jax reference: relative error 4.5e-3.

RK4 algebra per unfold step (dt = 1/6):
    d_j = scale * tanh(xb + v_j @ R) - v_j            (j = 1..4)
    v_1 = h;  v_2 = h + (dt/2) d_1;  v_3 = h + (dt/2) d_2;  v_4 = h + dt d_3
    h' = h + (dt/6) (d_1 + 2 d_2 + 2 d_3 + d_4)
The accumulation is built progressively (acc alias + in-place scaling) so
each d tile is released after its stage and the step-end critical path is
two DVE ops.

Engine split: PE = matmuls + 128x128 transposes + xb injection; ACT = tanh
and half the per-partition scale applications (Tanh/Copy/Identity share
one activation table -- no reload thrash); DVE = PSUM evacuations and
fused tensor_scalar/tensor_tensor combines (scalar_tensor_tensor is
avoided: it has no DVE fast mode, and is illegal on Pool); Pool = shadow
copies and off-critical-path accumulation adds.

Performance: TimelineSim (cost model) 1.21 ms/core; real-hardware
differential measurement (30-step vs 6-step builds) ~0.97 ms/core, down
from 4.39 ms for the naive chunk-sequential fp32r version. Engine busy:
PE ~0.94 ms (near the 0.88 ms floor for this matmul count), DVE ~0.82 ms,
ACT ~0.75 ms.
"""

import numpy as np

_B, _DIN, _UNITS = 32768, 256, 512
_NCORES = 8
_BLOCAL = _B // _NCORES      # 4096
_CHUNK = 512                 # batch columns per chunk (one fp32 PSUM bank)
_NCHUNKS = _BLOCAL // _CHUNK # 8
_NSTEPS = 6
_DT = 1.0 / _NSTEPS

_cached = {}


def _build_program(n_chunks=_NCHUNKS, n_steps=_NSTEPS, state_dt="mixed", group=4, d_dt="bf16",
                   use_pool=True, act_evac=True, id_mm=True):
    from contextlib import ExitStack

    import concourse.bass as bass
    import concourse.tile as tile
    from concourse import bacc, mybir
    from concourse.masks import make_identity

    f32 = mybir.dt.float32
    f32r = mybir.dt.float32r
    bf16 = mybir.dt.bfloat16
    # Per-tensor dtype mix. Error decomposition (real inputs, numpy model):
    # h-state in bf16 costs 8e-3 relative, weights 2.6e-3, xb 2.1e-3,
    # d 1.9e-3, v/u < 1e-3. "mixed" keeps h/weights/xb in f32r (the PE rate
    # is set by the moving operand, so f32r weights are free) and v/u/d in
    # bf16 so the hot DVE ops keep their 2x/4x fast modes.
    cfgs = {
        "f32r":  dict(w=f32r, h=f32r, xb=f32r, vn=f32r, u=f32),
        "bf16":  dict(w=bf16, h=bf16, xb=bf16, vn=bf16, u=bf16),
        # h accumulates in f32 (it feeds no matmuls -- a bf16 shadow serves
        # those and the element-wise ops), killing the dominant 8e-3
        # h-storage quantization error while every matmul stays pure bf16
        "mixed": dict(w=bf16, h=f32, xb=bf16, vn=bf16, u=bf16),
    }
    cfg = cfgs[state_dt]
    DT_W, DT_H, DT_XB, DT_VN, DT_U = (
        cfg["w"], cfg["h"], cfg["xb"], cfg["vn"], cfg["u"]
    )
    # shadow: bf16 copy of h for element-wise consumers when h is f32r but
    # the work dtypes are bf16
    use_shadow = DT_H != DT_VN
    DT_D = {"f32": f32, "bf16": bf16}[d_dt]
    Alu = mybir.AluOpType
    Act = mybir.ActivationFunctionType

    UB = _UNITS // 128   # 4 unit blocks
    DB = _DIN // 128     # 2 d_in blocks
    BB = _CHUNK // 128   # 4 batch blocks per chunk
    W = UB * _CHUNK      # 2048: fused free width (4 unit-blocks side by side)
    WX = DB * _CHUNK     # 1024: fused width for x-transposed

    b_rows = n_chunks * _CHUNK
    assert n_chunks % group == 0

    nc = bacc.Bacc("TRN2", target_bir_lowering=False, debug=False)

    x_d = nc.dram_tensor("x", [b_rows, _DIN], f32, kind="ExternalInput")
    h_d = nc.dram_tensor("h0", [b_rows, _UNITS], f32, kind="ExternalInput")
    K_d = nc.dram_tensor("Kw", [_DIN, _UNITS], f32, kind="ExternalInput")
    R_d = nc.dram_tensor("Rw", [_UNITS, _UNITS], f32, kind="ExternalInput")
    b_d = nc.dram_tensor("bv", [_UNITS], f32, kind="ExternalInput")
    s_d = nc.dram_tensor("sv", [_UNITS], f32, kind="ExternalInput")
    o_d = nc.dram_tensor("out", [b_rows, _UNITS], f32, kind="ExternalOutput")

    with tile.TileContext(nc) as tc, ExitStack() as ctx:
        wpool = ctx.enter_context(tc.tile_pool(name="w", bufs=1))
        stgpool = ctx.enter_context(tc.tile_pool(name="stg", bufs=2))
        iopool = ctx.enter_context(tc.tile_pool(name="io", bufs=2))
        spool = ctx.enter_context(tc.tile_pool(name="state", bufs=group))
        hpool = ctx.enter_context(tc.tile_pool(name="hstate", bufs=2 * group))
        upool = ctx.enter_context(tc.tile_pool(name="u", bufs=2))
        # the f32 h-accumulator of "mixed" costs ~24KB/partition extra; pay
        # for it with slightly tighter intermediate pools
        tight = 1 if use_shadow else 2
        dpool = ctx.enter_context(tc.tile_pool(name="d", bufs=group + tight))
        apool = ctx.enter_context(tc.tile_pool(name="acc", bufs=2 + tight))
        shpool = ctx.enter_context(tc.tile_pool(name="hsh", bufs=group + 1))
        vpool = ctx.enter_context(tc.tile_pool(name="vn", bufs=group + tight))
        opool = ctx.enter_context(tc.tile_pool(name="o", bufs=1 + tight))
        pspool = ctx.enter_context(tc.tile_pool(name="ps", bufs=8, space="PSUM"))

        # ---- weights / constants (loaded once; rounded via DVE copies) ----
        R_sb = []
        for kb in range(UB):
            stg = stgpool.tile([128, _UNITS], f32, tag="stg")
            nc.sync.dma_start(out=stg[:], in_=R_d[kb * 128:(kb + 1) * 128, :])
            t = wpool.tile([128, _UNITS], DT_W, tag=f"R{kb}")
            nc.vector.tensor_copy(t[:], stg[:])
            R_sb.append(t)
        K_sb = []
        for db in range(DB):
            stg = stgpool.tile([128, _UNITS], f32, tag="stg")
            nc.sync.dma_start(out=stg[:], in_=K_d[db * 128:(db + 1) * 128, :])
            t = wpool.tile([128, _UNITS], DT_W, tag=f"K{db}")
            nc.vector.tensor_copy(t[:], stg[:])
            K_sb.append(t)
        bias_sb = wpool.tile([128, UB], f32, tag="bias")
        nc.sync.dma_start(out=bias_sb[:], in_=b_d[:].rearrange("(j p) -> p j", p=128))
        scale_sb = wpool.tile([128, UB], f32, tag="scale")
        nc.sync.dma_start(out=scale_sb[:], in_=s_d[:].rearrange("(j p) -> p j", p=128))
        # f32 identity for the input/output transposes
        ident = wpool.tile([128, 128], f32, tag="ident")
        make_identity(nc, ident[:])
        # state-dtype identity for the xb PSUM injection (produced by a DVE
        # copy so the fp32r-rounding rule is satisfied)
        identW = wpool.tile([128, 128], DT_XB, tag="identW")
        nc.vector.tensor_copy(identW[:], ident[:])

        def mm(ps_ap, lhsT_ap, rhs_ap, start, stop):
            nc.tensor.matmul(ps_ap, lhsT_ap, rhs_ap, start=start, stop=stop)

        for g0 in range(0, n_chunks, group):
            chunks = list(range(g0, g0 + group))
            st = {c: {} for c in chunks}

            for c in chunks:
                r0 = c * _CHUNK

                # ---- load chunk in natural layout ----
                xn, hn = [], []
                for bb in range(BB):
                    t = iopool.tile([128, _DIN], f32, tag=f"xn{bb}")
                    nc.sync.dma_start(
                        out=t[:], in_=x_d[r0 + bb * 128:r0 + (bb + 1) * 128, :]
                    )
                    xn.append(t)
                for bb in range(BB):
                    t = iopool.tile([128, _UNITS], f32, tag=f"hn{bb}")
                    nc.sync.dma_start(
                        out=t[:], in_=h_d[r0 + bb * 128:r0 + (bb + 1) * 128, :]
                    )
                    hn.append(t)

                # ---- transpose x chunk -> xT ----
                xT = spool.tile([128, WX], DT_W, tag="xT")
                for db in range(DB):
                    ps = pspool.tile([128, _CHUNK], f32, tag="ps")
                    for bb in range(BB):
                        nc.tensor.transpose(
                            ps[:, bb * 128:(bb + 1) * 128],
                            xn[bb][:, db * 128:(db + 1) * 128],
                            ident[:],
                        )
                    nc.vector.tensor_copy(xT[:, db * _CHUNK:(db + 1) * _CHUNK], ps[:])

                # ---- transpose h chunk -> hT ----
                hT = hpool.tile([128, W], DT_H, tag="hT")
                for ub in range(UB):
                    ps = pspool.tile([128, _CHUNK], f32, tag="ps")
                    for bb in range(BB):
                        nc.tensor.transpose(
                            ps[:, bb * 128:(bb + 1) * 128],
                            hn[bb][:, ub * 128:(ub + 1) * 128],
                            ident[:],
                        )
                    (nc.scalar.copy if act_evac else nc.vector.tensor_copy)(hT[:, ub * _CHUNK:(ub + 1) * _CHUNK], ps[:])
                st[c]["hT"] = hT
                if use_shadow:
                    hsh = shpool.tile([128, W], DT_VN, tag="hsh")
                    nc.gpsimd.tensor_copy(hsh[:], hT[:])
                    st[c]["hsh"] = hsh

                # ---- xbT = (x @ K).T + bias ----
                xbT = spool.tile([128, W], DT_XB, tag="xbT")
                for ub in range(UB):
                    ps = pspool.tile([128, _CHUNK], f32, tag="ps")
                    for db in range(DB):
                        mm(
                            ps[:],
                            K_sb[db][:, ub * 128:(ub + 1) * 128],
                            xT[:, db * _CHUNK:(db + 1) * _CHUNK],
                            start=(db == 0),
                            stop=(db == DB - 1),
                        )
                    nc.vector.tensor_scalar_add(
                        xbT[:, ub * _CHUNK:(ub + 1) * _CHUNK],
                        ps[:],
                        bias_sb[:, ub:ub + 1],
                    )
                st[c]["xbT"] = xbT
                st[c]["vcur"] = st[c]["hsh"] if use_shadow else hT

            # ---- RK4 unfold steps, chunk-group interleaved per stage ----
            for s in range(n_steps):
                for j in range(4):
                    # matmul groups: weight-stationary order (ub, kb, chunk);
                    # xb lands first via the identity weight (start=True)
                    for c in chunks:
                        st[c]["ps"] = [
                            pspool.tile([128, _CHUNK], f32, tag="ps", name="ps")
                            for _ in range(UB)
                        ]
                    for ub in range(UB):
                        if id_mm:
                            for c in chunks:
                                mm(
                                    st[c]["ps"][ub][:],
                                    identW[:],
                                    st[c]["xbT"][:, ub * _CHUNK:(ub + 1) * _CHUNK],
                                    start=True,
                                    stop=False,
                                )
                        for kb in range(UB):
                            for c in chunks:
                                mm(
                                    st[c]["ps"][ub][:],
                                    R_sb[kb][:, ub * 128:(ub + 1) * 128],
                                    st[c]["vcur"][:, kb * _CHUNK:(kb + 1) * _CHUNK],
                                    start=(not id_mm and kb == 0),
                                    stop=(kb == UB - 1),
                                )

                    for c in chunks:
                        hT = st[c]["hT"]
                        vcur = st[c]["vcur"]
                        # u = tanh(psum); then u *= scale in place (half the
                        # unit blocks on ACT, half on DVE)
                        u = upool.tile([128, W], DT_U, tag="u")
                        for ub in range(UB):
                            usl = u[:, ub * _CHUNK:(ub + 1) * _CHUNK]
                            if id_mm:
                                nc.scalar.activation(usl, st[c]["ps"][ub][:], Act.Tanh)
                            else:
                                nc.vector.tensor_add(
                                    usl, st[c]["ps"][ub][:],
                                    st[c]["xbT"][:, ub * _CHUNK:(ub + 1) * _CHUNK],
                                )
                                nc.scalar.activation(usl, usl, Act.Tanh)
                        for ub in range(UB):
                            sl = u[:, ub * _CHUNK:(ub + 1) * _CHUNK]
                            if ub % 2 == 0:
                                nc.scalar.activation(
                                    sl, sl, Act.Identity,
                                    scale=scale_sb[:, ub:ub + 1],
                                )
                            else:
                                nc.vector.tensor_scalar_mul(
                                    sl, sl, scale_sb[:, ub:ub + 1]
                                )
                        # d = u - vcur
                        d = dpool.tile([128, W], DT_D, tag="d")
                        nc.vector.tensor_sub(d[:], u[:], vcur[:])
                        # next stage input: v = h + c_j * d (before d is
                        # scaled in place for the RK4 accumulation)
                        if j < 3:
                            cj = _DT / 2.0 if j < 2 else _DT
                            helem = st[c]["hsh"] if use_shadow else hT
                            vn = vpool.tile([128, W], DT_VN, tag="vn")
                            nc.vector.tensor_scalar_mul(vn[:], d[:], cj)
                            nc.vector.tensor_add(vn[:], vn[:], helem[:])
                            st[c]["vcur"] = vn
                        # progressive RK4 accumulation: acc = d1 + 2 d2 + 2 d3
                        # + d4, built per stage so each d is released
                        # immediately and the step-end critical path is short
                        if j == 0:
                            acc = apool.tile([128, W], DT_D, tag="acc")
                            (nc.gpsimd if use_pool else nc.vector).tensor_copy(acc[:], d[:])
                            st[c]["acc"] = acc
                        else:
                            acc = st[c]["acc"]
                            if j < 3:
                                nc.vector.tensor_scalar_mul(d[:], d[:], 2.0)
                                (nc.gpsimd if use_pool else nc.vector).tensor_add(acc[:], acc[:], d[:])
                            else:
                                nc.vector.tensor_add(acc[:], acc[:], d[:])
                        if j == 3:
                            # h' = h + dt/6 * acc
                            nc.vector.tensor_scalar_mul(acc[:], acc[:], _DT / 6.0)
                            hdt = f32 if s == n_steps - 1 else DT_H
                            hnew = hpool.tile([128, W], hdt, tag="hT")
                            nc.vector.tensor_add(hnew[:], acc[:], hT[:])
                            st[c]["hT"] = hnew
                            if use_shadow and s < n_steps - 1:
                                hsh = shpool.tile([128, W], DT_VN, tag="hsh")
                                nc.gpsimd.tensor_copy(hsh[:], hnew[:])
                                st[c]["hsh"] = hsh
                                st[c]["vcur"] = hsh
                            else:
                                st[c]["vcur"] = hnew

            # ---- transpose h back to natural layout and store ----
            for c in chunks:
                r0 = c * _CHUNK
                hT = st[c]["hT"]
                for bb in range(BB):
                    ps = pspool.tile([128, _CHUNK], f32, tag="ps")
                    for ub in range(UB):
                        nc.tensor.transpose(
                            ps[:, ub * 128:(ub + 1) * 128],
                            hT[:, ub * _CHUNK + bb * 128:ub * _CHUNK + (bb + 1) * 128],
                            ident[:],
                        )
                    o_sb = opool.tile([128, _UNITS], f32, tag="o")
                    (nc.scalar.copy if act_evac else nc.vector.tensor_copy)(o_sb[:], ps[:])
                    nc.sync.dma_start(
                        out=o_d[r0 + bb * 128:r0 + (bb + 1) * 128, :],
                        in_=o_sb[:],
                    )

    nc.compile()
    return nc


def _get_program():
    if "nc" not in _cached:
        _cached["nc"] = _build_program()
    return _cached["nc"]


def _make_in_maps(inputs, hidden_state, kern, recurrent_kernel, bias, scale):
    def f(a):
        return np.ascontiguousarray(np.asarray(a), dtype=np.float32)

    x = f(inputs)
    h = f(hidden_state)
    shared = {
        "Kw": f(kern),
        "Rw": f(recurrent_kernel),
        "bv": f(bias),
        "sv": f(scale),
    }
    maps = []
    for c in range(_NCORES):
        sl = slice(c * _BLOCAL, (c + 1) * _BLOCAL)
        maps.append({"x": x[sl], "h0": h[sl], **shared})
    return maps


def _run(in_maps, trace=False):
    from concourse.bass_utils import run_bass_kernel_spmd

    nc = _get_program()
    res = run_bass_kernel_spmd(nc, in_maps, list(range(_NCORES)), trace=trace)
    out = np.concatenate(
        [res.results[i]["out"] for i in range(_NCORES)], axis=0
    ).astype(np.float32)
    return out, res


def kernel(inputs, hidden_state, kernel, recurrent_kernel, bias, scale):
    in_maps = _make_in_maps(inputs, hidden_state, kernel, recurrent_kernel, bias, scale)
    out, _ = _run(in_maps, trace=False)
    return out

